# revision 5
# baseline (speedup 1.0000x reference)
"""Trainium2 Bass kernel for nn_AllLoss_13400297964003.

Strategy (exact algebraic refactor of the reference loss):
  - The mask BCE term per anchor m is
        mean_{512x512}( softplus(up) - goal*up )
    with up = 4x nearest-upsample of z_m = coef_m . proto.  This equals
        ( 16*sum_ij softplus(z_m[ij]) - sum_ij z_m[ij]*G_m[ij] ) / 512^2
    where G_m = 4x4 block-sum pooling of gt_masks[gt_idx[m]].
  - The goal term collapses:  sum_m sum_ij z_m*G_m = sum_{p,g} C[p,g]*D[p,g]
    with C[p,g] = sum_{m: gt_idx[m]=g} coef[m,p]  (tiny, host-aggregated)
    and  D[p,g] = sum_ij proto[p,ij] * pool4x4(mask_g)[ij]  (device).
  - Sharding over 8 cores: core c gets anchors [32c,32c+32), gt masks
    [4c,4c+4), and 96 negative anchors.  Each core reads 4.2MB of masks
    (a perfect shard of the 33.5MB dominant input), computes partial sums,
    host combines scalars in float64.

Device work per core:
  - z via one float32r matmul chain: block-diag weights [16,128] x
    proto16 [16,4096] -> z in PSUM [128 x 4096] (8 banks), full partition
    occupancy.  softplus = Exp then Ln(bias=1) on ACT with accum_out.
  - mask 4x4 pooling: row-pool via float32r matmuls with constant 0/1
    matrices (exact for 0/1 masks), column-pool via strided DVE adds.
  - D partials via DVE multiply + segmented reduce.
  - cls/loc losses as packed 128-row columns (gathers done host-side,
    all arithmetic incl. log10/reciprocal/smooth-L1 on device).
"""

import numpy as np

N_CORES = 8
M = 256
NUM_GT = 32
M_LOC = M // N_CORES          # 32 anchors per core
G_LOC = NUM_GT // N_CORES     # 4 gt masks per core
NEG_LOC = 3 * M // N_CORES    # 96 negative anchors per core
LN10 = float(np.log(10.0))
NCOL = 26                     # result columns: 8 soft, 1 cls, 1 loc, 16 ddot

_CACHE = {}


def _build_nc():
    from contextlib import ExitStack
    import concourse.tile as tile
    from concourse import bacc, mybir

    f32 = mybir.dt.float32
    f32r = mybir.dt.float32r
    AF = mybir.ActivationFunctionType
    ALU = mybir.AluOpType
    AX = mybir.AxisListType

    nc = bacc.Bacc("TRN2", target_bir_lowering=False, debug=False)

    masks = nc.dram_tensor("masks", [G_LOC, 512, 512], f32r, kind="ExternalInput").ap()
    proto16 = nc.dram_tensor("proto16", [16, 4096], f32r, kind="ExternalInput").ap()
    proto_cat = nc.dram_tensor("proto_cat", [128, 512], f32, kind="ExternalInput").ap()
    w16 = nc.dram_tensor("w16", [16, 128], f32r, kind="ExternalInput").ap()
    sr = nc.dram_tensor("sr", [128, 512], f32r, kind="ExternalInput").ap()
    clsx = nc.dram_tensor("clsx", [128, 1], f32, kind="ExternalInput").ap()
    clssgn = nc.dram_tensor("clssgn", [128, 1], f32, kind="ExternalInput").ap()
    locp = nc.dram_tensor("locp", [128, 1], f32, kind="ExternalInput").ap()
    locu = nc.dram_tensor("locu", [128, 1], f32, kind="ExternalInput").ap()
    locv = nc.dram_tensor("locv", [128, 1], f32, kind="ExternalInput").ap()
    locw = nc.dram_tensor("locw", [128, 1], f32, kind="ExternalInput").ap()
    res = nc.dram_tensor("res", [128, NCOL], f32, kind="ExternalOutput").ap()

    with tile.TileContext(nc) as tc:
        with ExitStack() as ctx:
            constp = ctx.enter_context(tc.tile_pool(name="constp", bufs=1))
            maskp = ctx.enter_context(tc.tile_pool(name="maskp", bufs=16))
            zps = ctx.enter_context(tc.tile_pool(name="zps", bufs=4, space="PSUM"))
            rps = ctx.enter_context(tc.tile_pool(name="rps", bufs=4, space="PSUM"))
            workp = ctx.enter_context(tc.tile_pool(name="workp", bufs=3))
            outp = ctx.enter_context(tc.tile_pool(name="outp", bufs=1))

            # ---- constant / small input loads ----
            proto16_t = constp.tile([16, 4096], f32r)
            nc.sync.dma_start(proto16_t[:], proto16[:])
            w16_t = constp.tile([16, 128], f32r)
            nc.sync.dma_start(w16_t[:], w16[:])
            sr_t = constp.tile([128, 512], f32r)
            nc.sync.dma_start(sr_t[:], sr[:])
            proto_cat_t = constp.tile([128, 512], f32)
            nc.sync.dma_start(proto_cat_t[:], proto_cat[:])
            clsx_t = constp.tile([128, 1], f32)
            nc.sync.dma_start(clsx_t[:], clsx[:])
            clssgn_t = constp.tile([128, 1], f32)
            nc.sync.dma_start(clssgn_t[:], clssgn[:])
            locp_t = constp.tile([128, 1], f32)
            nc.sync.dma_start(locp_t[:], locp[:])
            locu_t = constp.tile([128, 1], f32)
            nc.sync.dma_start(locu_t[:], locu[:])
            locv_t = constp.tile([128, 1], f32)
            nc.sync.dma_start(locv_t[:], locv[:])
            locw_t = constp.tile([128, 1], f32)
            nc.sync.dma_start(locw_t[:], locw[:])

            PS = outp.tile([128, NCOL], f32)

            # ---- mask chunk DMAs (the dominant traffic) ----
            chunk = {}
            for g in range(G_LOC):
                for c in range(4):
                    t = maskp.tile([128, 512], f32r, tag="mask")
                    nc.sync.dma_start(t[:], masks[g, 128 * c:128 * (c + 1), :])
                    chunk[(g, c)] = t

            # ---- z matmul + softplus accumulation ----
            w16r = w16_t[:]
            p16r = proto16_t[:]
            for b in range(8):
                zt = zps.tile([128, 512], f32, tag="z")
                nc.tensor.matmul(zt[:], w16r, p16r[:, 512 * b:512 * (b + 1)],
                                 start=True, stop=True)
                ex = workp.tile([128, 512], f32, tag="ex")
                nc.scalar.activation(ex[:], zt[:], AF.Exp)
                nc.scalar.activation(ex[:], ex[:], AF.Ln, bias=1.0,
                                     accum_out=PS[:, b:b + 1])

            # ---- mask pooling + D partials ----
            pc3 = proto_cat_t[:].rearrange("p (a k) -> p a k", a=4)
            for g in range(G_LOC):
                R = rps.tile([128, 512], f32, tag="r")
                for c in range(4):
                    nc.tensor.matmul(
                        R[:],
                        sr_t[:, 128 * c:128 * (c + 1)],
                        chunk[(g, c)][:],
                        start=(c == 0), stop=(c == 3),
                    )
                r4 = R[:].rearrange("p (j four) -> p j four", four=4)
                Pg = workp.tile([128, 128], f32, tag="Pg")
                nc.vector.tensor_reduce(Pg[:], r4, axis=AX.X, op=ALU.add)
                prod = workp.tile([128, 4, 128], f32, tag="prod")
                pgb = Pg[:].unsqueeze(1).broadcast_to([128, 4, 128])
                nc.vector.tensor_mul(prod[:], pgb, pc3)
                nc.vector.tensor_reduce(PS[:, 10 + 4 * g:14 + 4 * g], prod[:],
                                        axis=AX.X, op=ALU.add)

            # ---- classification loss column ----
            et = workp.tile([128, 1], f32, tag="sm1")
            nc.scalar.activation(et[:], clsx_t[:], AF.Exp, scale=clssgn_t[:])
            nc.scalar.activation(PS[:, 8:9], et[:], AF.Ln, bias=1.0)

            # ---- localization loss column ----
            fu = workp.tile([128, 1], f32, tag="sm2")
            nc.scalar.activation(fu[0:64, :], locu_t[0:64, :], AF.Identity)
            nc.scalar.activation(fu[64:128, :], locu_t[64:128, :], AF.Ln)
            fv = workp.tile([128, 1], f32, tag="sm3")
            nc.scalar.activation(fv[0:64, :], locv_t[0:64, :], AF.Identity)
            nc.scalar.activation(fv[64:128, :], locv_t[64:128, :], AF.Ln)
            rw = workp.tile([128, 1], f32, tag="sm4")
            nc.vector.reciprocal(rw[:], locw_t[:])
            df = workp.tile([128, 1], f32, tag="sm5")
            nc.vector.tensor_sub(df[:], fu[:], fv[:])
            tgt = workp.tile([128, 1], f32, tag="sm6")
            nc.vector.tensor_mul(tgt[:], df[:], rw[:])
            d = workp.tile([128, 1], f32, tag="sm7")
            nc.vector.tensor_sub(d[:], locp_t[:], tgt[:])
            # smooth-l1: a=|d|, m=min(a,1), f = 0.5*m^2 + a - m
            a_t = workp.tile([128, 1], f32, tag="sm8")
            nc.scalar.activation(a_t[:], d[:], AF.Abs)
            mn = workp.tile([128, 1], f32, tag="sm9")
            nc.vector.tensor_scalar(mn[:], a_t[:], 1.0, None, op0=ALU.min)
            amn = workp.tile([128, 1], f32, tag="sm10")
            nc.vector.tensor_sub(amn[:], a_t[:], mn[:])
            sq = workp.tile([128, 1], f32, tag="sm11")
            nc.vector.tensor_mul(sq[:], mn[:], mn[:])
            nc.vector.scalar_tensor_tensor(PS[:, 9:10], sq[:], 0.5, amn[:],
                                           op0=ALU.mult, op1=ALU.add)

            # ---- write result ----
            nc.sync.dma_start(res[:], PS[:])

    nc.compile()
    return nc


def _get_nc():
    if "nc" not in _CACHE:
        _CACHE["nc"] = _build_nc()
    return _CACHE["nc"]


def _host_prep(inputs):
    """Pure index-driven gathers/packing. Returns per-core input maps plus
    the float64 C aggregation matrix used in the final scalar combine."""
    f32 = np.float32
    proto = np.asarray(inputs["proto_types"], f32)[0]        # (4,128,128)
    map_class = np.asarray(inputs["map_class"], f32)[0]      # (3,64,64)
    map_box = np.asarray(inputs["map_box"], f32)[0]          # (12,64,64)
    map_coef = np.asarray(inputs["map_coef"], f32)[0]        # (12,64,64)
    anchor_center = np.asarray(inputs["anchor_center"], f32)  # (2,64,64)
    anchor_box = np.asarray(inputs["anchor_box"], f32)       # (3,2)
    gt_boxes = np.asarray(inputs["gt_boxes"], f32)[0]        # (32,4)
    gt_masks = np.asarray(inputs["gt_masks"], f32)[0]        # (32,512,512)
    pos_idx = np.asarray(inputs["pos_idx"])
    gt_idx = np.asarray(inputs["gt_idx"])
    neg_idx = np.asarray(inputs["neg_idx"])

    r, hh, ww = pos_idx[:, 0], pos_idx[:, 1], pos_idx[:, 2]
    ch4 = r[:, None] * 4 + np.arange(4, dtype=r.dtype)[None, :]
    coef = map_coef[ch4, hh[:, None], ww[:, None]]           # (256,4)
    pred = map_box[ch4, hh[:, None], ww[:, None]]            # (256,4)
    logit_pos = map_class[r, hh, ww]                         # (256,)
    logit_neg = map_class[neg_idx[:, 0], neg_idx[:, 1], neg_idx[:, 2]]  # (768,)
    a_ch = anchor_center[0, hh, ww]
    a_cw = anchor_center[1, hh, ww]
    a_h = anchor_box[r, 0]
    a_w = anchor_box[r, 1]
    gt = gt_boxes[gt_idx]                                    # (256,4)

    # replicated tensors
    proto_flat = proto.reshape(4, 16384)
    proto16 = np.ascontiguousarray(
        proto_flat.reshape(4, 4, 4096).transpose(1, 0, 2).reshape(16, 4096))
    proto_cat = np.ascontiguousarray(proto.transpose(1, 0, 2).reshape(128, 512))
    sr = np.zeros((128, 512), f32)
    for c in range(4):
        for I in range(128):
            sr[I, 128 * c + 32 * c + I // 4] = 1.0

    # C[p,g] aggregation (float64, host)
    C = np.zeros((4, NUM_GT), np.float64)
    for p in range(4):
        np.add.at(C[p], gt_idx, coef[:, p].astype(np.float64))

    in_maps = []
    for cidx in range(N_CORES):
        msel = slice(M_LOC * cidx, M_LOC * (cidx + 1))
        nsel = slice(NEG_LOC * cidx, NEG_LOC * (cidx + 1))
        coef_c = coef[msel]                                  # (32,4)
        w16 = np.zeros((16, 128), f32)
        for q in range(4):
            w16[4 * q:4 * q + 4, 32 * q:32 * q + 32] = coef_c.T
        clsx = np.concatenate([logit_pos[msel], logit_neg[nsel]]).reshape(128, 1)
        clssgn = np.concatenate([
            np.full(M_LOC, -1.0, f32), np.full(NEG_LOC, 1.0, f32)]).reshape(128, 1)
        # k-blocked loc packing: rows k*32 + j
        pr = pred[msel]                                      # (32,4)
        gtc = gt[msel]
        locp_a = np.ascontiguousarray(pr.T.reshape(128, 1))
        locu_a = np.ascontiguousarray(gtc.T.reshape(128, 1))
        locv_a = np.concatenate(
            [a_ch[msel], a_cw[msel], a_h[msel], a_w[msel]]).reshape(128, 1)
        locw_a = np.concatenate(
            [a_h[msel], a_w[msel],
             np.full(M_LOC, LN10, f32), np.full(M_LOC, LN10, f32)]).reshape(128, 1)
        in_maps.append({
            "masks": np.ascontiguousarray(gt_masks[G_LOC * cidx:G_LOC * (cidx + 1)]),
            "proto16": proto16,
            "proto_cat": proto_cat,
            "w16": w16,
            "sr": sr,
            "clsx": np.ascontiguousarray(clsx, dtype=f32),
            "clssgn": clssgn.astype(f32),
            "locp": locp_a.astype(f32),
            "locu": locu_a.astype(f32),
            "locv": np.ascontiguousarray(locv_a, dtype=f32),
            "locw": np.ascontiguousarray(locw_a, dtype=f32),
        })
    return in_maps, C


def _combine(results, C):
    """results: list of per-core {'res': [128, NCOL]} dicts. float64 combine."""
    s_soft = 0.0
    s_cls = 0.0
    s_loc = 0.0
    s_dot = 0.0
    for cidx in range(N_CORES):
        rc = np.asarray(results[cidx]["res"], np.float64)
        s_soft += rc[:, 0:8].sum()
        s_cls += rc[:, 8].sum()
        s_loc += rc[:, 9].sum()
        for g in range(G_LOC):
            for p in range(4):
                s_dot += C[p, G_LOC * cidx + g] * rc[:, 10 + 4 * g + p].sum()
    total = s_cls + s_loc + (16.0 * s_soft - s_dot) / 262144.0 / float(M)
    return np.array(total, dtype=np.float32)


def kernel(**inputs):
    from concourse.bass_utils import run_bass_kernel_spmd
    nc = _get_nc()
    in_maps, C = _host_prep(inputs)
    out = run_bass_kernel_spmd(nc, in_maps, list(range(N_CORES)))
    return _combine(out.results, C)


# revision 10
# speedup vs baseline: 1.1939x; 1.1939x over previous
"""Trainium2 Bass kernel for nn_AllLoss_13400297964003.

Strategy (exact algebraic refactor of the reference loss):
  - The mask BCE term per anchor m is
        mean_{512x512}( softplus(up) - goal*up )
    with up = 4x nearest-upsample of z_m = coef_m . proto.  This equals
        ( 16*sum_ij softplus(z_m[ij]) - sum_ij z_m[ij]*G_m[ij] ) / 512^2
    where G_m = 4x4 block-sum pooling of gt_masks[gt_idx[m]].
  - The goal term collapses:  sum_m sum_ij z_m*G_m = sum_{p,g} C[p,g]*D[p,g]
    with C[p,g] = sum_{m: gt_idx[m]=g} coef[m,p]  (tiny, host-aggregated)
    and  D[p,g] = sum_ij proto[p,ij] * pool4x4(mask_g)[ij]  (device).
  - Sharding over 8 cores: core c gets anchors [32c,32c+32), gt masks
    [4c,4c+4), and 96 negative anchors.  Each core reads 4.2MB of masks
    (a perfect shard of the 33.5MB dominant input), computes partial sums,
    host combines scalars in float64.

Device work per core:
  - z via float32r matmuls: block-diag weights [16,128] x proto16
    [16,4096] -> z in PSUM (full partition occupancy).  softplus = Exp
    then Ln(bias=1) on ACT with accum_out; all Exp-set ops are
    chain-ordered before all Ln-set ops so the ACT spline tables load
    exactly twice.
  - mask 4x4 pooling: row-pool via float32r matmuls with constant 0/1
    matrices (exact for 0/1 masks), column-pool via one strided
    tensor_reduce per mask.
  - D partials via DVE multiply + segmented reduce.
  - cls/loc losses as packed 128-row columns (gathers done host-side,
    all arithmetic incl. log10/reciprocal/smooth-L1 on device).
"""

import numpy as np

N_CORES = 8
M = 256
NUM_GT = 32
M_LOC = M // N_CORES          # 32 anchors per core
G_LOC = NUM_GT // N_CORES     # 4 gt masks per core
NEG_LOC = 3 * M // N_CORES    # 96 negative anchors per core
LN10 = float(np.log(10.0))
NCOL = 26                     # result cols: 8 soft, 1 cls, 1 loc, 16 ddot

_CACHE = {}


def _build_nc():
    from contextlib import ExitStack
    import concourse.tile as tile
    from concourse import bacc, mybir
    from concourse.tile import add_dep_helper

    f32 = mybir.dt.float32
    f32r = mybir.dt.float32r
    AF = mybir.ActivationFunctionType
    ALU = mybir.AluOpType
    AX = mybir.AxisListType

    nc = bacc.Bacc("TRN2", target_bir_lowering=False, debug=False)

    masks = nc.dram_tensor("masks", [G_LOC, 512, 512], f32r, kind="ExternalInput").ap()
    proto16 = nc.dram_tensor("proto16", [16, 4096], f32r, kind="ExternalInput").ap()
    proto_cat = nc.dram_tensor("proto_cat", [128, 512], f32, kind="ExternalInput").ap()
    w16 = nc.dram_tensor("w16", [16, 128], f32r, kind="ExternalInput").ap()
    sr = nc.dram_tensor("sr", [128, 512], f32r, kind="ExternalInput").ap()
    clsx = nc.dram_tensor("clsx", [128, 1], f32, kind="ExternalInput").ap()
    clssgn = nc.dram_tensor("clssgn", [128, 1], f32, kind="ExternalInput").ap()
    locp = nc.dram_tensor("locp", [128, 1], f32, kind="ExternalInput").ap()
    locu = nc.dram_tensor("locu", [128, 1], f32, kind="ExternalInput").ap()
    locv = nc.dram_tensor("locv", [128, 1], f32, kind="ExternalInput").ap()
    locw = nc.dram_tensor("locw", [128, 1], f32, kind="ExternalInput").ap()
    res = nc.dram_tensor("res", [128, NCOL], f32, kind="ExternalOutput").ap()

    with tile.TileContext(nc) as tc:
        with ExitStack() as ctx:
            constp = ctx.enter_context(tc.tile_pool(name="constp", bufs=1))
            maskp = ctx.enter_context(tc.tile_pool(name="maskp", bufs=16))
            zps = ctx.enter_context(tc.tile_pool(name="zps", bufs=4, space="PSUM"))
            rps = ctx.enter_context(tc.tile_pool(name="rps", bufs=4, space="PSUM"))
            exps = ctx.enter_context(tc.tile_pool(name="exps", bufs=8))
            workp = ctx.enter_context(tc.tile_pool(name="workp", bufs=3))
            outp = ctx.enter_context(tc.tile_pool(name="outp", bufs=1))

            # ---- constant / small input loads ----
            sr_t = constp.tile([128, 512], f32r)
            nc.sync.dma_start(sr_t[:], sr[:])
            proto16_t = constp.tile([16, 4096], f32r)
            nc.sync.dma_start(proto16_t[:], proto16[:])
            w16_t = constp.tile([16, 128], f32r)
            nc.sync.dma_start(w16_t[:], w16[:])
            clsx_t = constp.tile([128, 1], f32)
            nc.sync.dma_start(clsx_t[:], clsx[:])
            clssgn_t = constp.tile([128, 1], f32)
            nc.sync.dma_start(clssgn_t[:], clssgn[:])
            locp_t = constp.tile([128, 1], f32)
            nc.sync.dma_start(locp_t[:], locp[:])
            locu_t = constp.tile([128, 1], f32)
            nc.sync.dma_start(locu_t[:], locu[:])
            locv_t = constp.tile([128, 1], f32)
            nc.sync.dma_start(locv_t[:], locv[:])
            locw_t = constp.tile([128, 1], f32)
            nc.sync.dma_start(locw_t[:], locw[:])

            # ---- mask chunk DMAs (the dominant traffic) ----
            chunk = {}
            for g in range(G_LOC):
                for c in range(4):
                    t = maskp.tile([128, 512], f32r, tag="mask")
                    nc.sync.dma_start(t[:], masks[g, 128 * c:128 * (c + 1), :])
                    chunk[(g, c)] = t

            proto_cat_t = constp.tile([128, 512], f32)
            nc.sync.dma_start(proto_cat_t[:], proto_cat[:])

            PS = outp.tile([128, NCOL], f32)

            exp_phase = []   # ACT ops using the Exp table set (+ fillers)
            ln_phase = []    # ACT ops using the Ln table set

            # ---- z matmuls (f32r) + softplus ----
            ex_tiles = []
            for b in range(8):
                zt = zps.tile([128, 512], f32, tag="z")
                nc.tensor.matmul(zt[:], w16_t[:],
                                 proto16_t[:, 512 * b:512 * (b + 1)],
                                 start=True, stop=True)
                ex = exps.tile([128, 512], f32, tag="ex")
                exp_phase.append(nc.scalar.activation(ex[:], zt[:], AF.Exp))
                ex_tiles.append(ex)

            # ---- cls/loc ACT ops, grouped into the two table phases ----
            et = workp.tile([128, 1], f32, tag="sm1")
            exp_phase.append(
                nc.scalar.activation(et[:], clsx_t[:], AF.Exp, scale=clssgn_t[:]))
            fu = workp.tile([128, 1], f32, tag="sm2")
            exp_phase.append(
                nc.scalar.activation(fu[0:64, :], locu_t[0:64, :], AF.Identity))
            fv = workp.tile([128, 1], f32, tag="sm3")
            exp_phase.append(
                nc.scalar.activation(fv[0:64, :], locv_t[0:64, :], AF.Identity))

            for b in range(8):
                ln_phase.append(
                    nc.scalar.activation(ex_tiles[b][:], ex_tiles[b][:], AF.Ln,
                                         bias=1.0, accum_out=PS[:, b:b + 1]))
            ln_phase.append(
                nc.scalar.activation(PS[:, 8:9], et[:], AF.Ln, bias=1.0))
            ln_phase.append(
                nc.scalar.activation(fu[64:128, :], locu_t[64:128, :], AF.Ln))
            ln_phase.append(
                nc.scalar.activation(fv[64:128, :], locv_t[64:128, :], AF.Ln))

            # chain the ACT program order: all Exp-set ops, then all Ln-set ops
            order = exp_phase + ln_phase
            for a, b2 in zip(order, order[1:]):
                add_dep_helper(b2.ins, a.ins, sync=False, reason="act-table-phase")

            # ---- mask pooling + D partials ----
            pc3 = proto_cat_t[:].rearrange("p (a k) -> p a k", a=4)
            for g in range(G_LOC):
                R = rps.tile([128, 512], f32, tag="r")
                for c in range(4):
                    nc.tensor.matmul(
                        R[:],
                        sr_t[:, 128 * c:128 * (c + 1)],
                        chunk[(g, c)][:],
                        start=(c == 0), stop=(c == 3),
                    )
                r4 = R[:].rearrange("p (j four) -> p j four", four=4)
                Pg = workp.tile([128, 128], f32, tag="Pg")
                nc.vector.tensor_reduce(Pg[:], r4, axis=AX.X, op=ALU.add)
                prod = workp.tile([128, 4, 128], f32, tag="prod")
                pgb = Pg[:].unsqueeze(1).broadcast_to([128, 4, 128])
                nc.vector.tensor_mul(prod[:], pgb, pc3)
                nc.vector.tensor_reduce(PS[:, 10 + 4 * g:14 + 4 * g], prod[:],
                                        axis=AX.X, op=ALU.add)

            # ---- localization smooth-L1 column ----
            rw = workp.tile([128, 1], f32, tag="sm4")
            nc.vector.reciprocal(rw[:], locw_t[:])
            df = workp.tile([128, 1], f32, tag="sm5")
            nc.vector.tensor_sub(df[:], fu[:], fv[:])
            tgt = workp.tile([128, 1], f32, tag="sm6")
            nc.vector.tensor_mul(tgt[:], df[:], rw[:])
            d = workp.tile([128, 1], f32, tag="sm7")
            nc.vector.tensor_sub(d[:], locp_t[:], tgt[:])
            a_t = workp.tile([128, 1], f32, tag="sm8")
            nc.scalar.activation(a_t[:], d[:], AF.Abs)
            mn = workp.tile([128, 1], f32, tag="sm9")
            nc.vector.tensor_scalar(mn[:], a_t[:], 1.0, None, op0=ALU.min)
            amn = workp.tile([128, 1], f32, tag="sm10")
            nc.vector.tensor_sub(amn[:], a_t[:], mn[:])
            sq = workp.tile([128, 1], f32, tag="sm11")
            nc.vector.tensor_mul(sq[:], mn[:], mn[:])
            nc.vector.scalar_tensor_tensor(PS[:, 9:10], sq[:], 0.5, amn[:],
                                           op0=ALU.mult, op1=ALU.add)

            # ---- write result ----
            nc.sync.dma_start(res[:], PS[:])

    nc.compile()
    return nc


def _get_nc():
    if "nc" not in _CACHE:
        _CACHE["nc"] = _build_nc()
    return _CACHE["nc"]


def _host_prep(inputs):
    """Pure index-driven gathers/packing. Returns per-core input maps plus
    the float64 C aggregation matrix used in the final scalar combine."""
    f32 = np.float32
    proto = np.asarray(inputs["proto_types"], f32)[0]        # (4,128,128)
    map_class = np.asarray(inputs["map_class"], f32)[0]      # (3,64,64)
    map_box = np.asarray(inputs["map_box"], f32)[0]          # (12,64,64)
    map_coef = np.asarray(inputs["map_coef"], f32)[0]        # (12,64,64)
    anchor_center = np.asarray(inputs["anchor_center"], f32)  # (2,64,64)
    anchor_box = np.asarray(inputs["anchor_box"], f32)       # (3,2)
    gt_boxes = np.asarray(inputs["gt_boxes"], f32)[0]        # (32,4)
    gt_masks = np.asarray(inputs["gt_masks"], f32)[0]        # (32,512,512)
    pos_idx = np.asarray(inputs["pos_idx"])
    gt_idx = np.asarray(inputs["gt_idx"])
    neg_idx = np.asarray(inputs["neg_idx"])

    r, hh, ww = pos_idx[:, 0], pos_idx[:, 1], pos_idx[:, 2]
    ch4 = r[:, None] * 4 + np.arange(4, dtype=r.dtype)[None, :]
    coef = map_coef[ch4, hh[:, None], ww[:, None]]           # (256,4)
    pred = map_box[ch4, hh[:, None], ww[:, None]]            # (256,4)
    logit_pos = map_class[r, hh, ww]                         # (256,)
    logit_neg = map_class[neg_idx[:, 0], neg_idx[:, 1], neg_idx[:, 2]]  # (768,)
    a_ch = anchor_center[0, hh, ww]
    a_cw = anchor_center[1, hh, ww]
    a_h = anchor_box[r, 0]
    a_w = anchor_box[r, 1]
    gt = gt_boxes[gt_idx]                                    # (256,4)

    # replicated tensors
    proto_flat = proto.reshape(4, 16384)
    proto16 = np.ascontiguousarray(
        proto_flat.reshape(4, 4, 4096).transpose(1, 0, 2).reshape(16, 4096))
    proto_cat = np.ascontiguousarray(proto.transpose(1, 0, 2).reshape(128, 512))
    sr = np.zeros((128, 512), f32)
    for c in range(4):
        for I in range(128):
            sr[I, 128 * c + 32 * c + I // 4] = 1.0

    # C[p,g] aggregation (float64, host)
    C = np.zeros((4, NUM_GT), np.float64)
    for p in range(4):
        np.add.at(C[p], gt_idx, coef[:, p].astype(np.float64))

    in_maps = []
    for cidx in range(N_CORES):
        msel = slice(M_LOC * cidx, M_LOC * (cidx + 1))
        nsel = slice(NEG_LOC * cidx, NEG_LOC * (cidx + 1))
        coef_c = coef[msel]                                  # (32,4)
        w16 = np.zeros((16, 128), f32)
        for q in range(4):
            w16[4 * q:4 * q + 4, 32 * q:32 * q + 32] = coef_c.T
        clsx_a = np.concatenate([logit_pos[msel], logit_neg[nsel]]).reshape(128, 1)
        clssgn_a = np.concatenate([
            np.full(M_LOC, -1.0, f32), np.full(NEG_LOC, 1.0, f32)]).reshape(128, 1)
        # k-blocked loc packing: rows k*32 + j
        locp_a = np.ascontiguousarray(pred[msel].T.reshape(128, 1))
        locu_a = np.ascontiguousarray(gt[msel].T.reshape(128, 1))
        locv_a = np.concatenate(
            [a_ch[msel], a_cw[msel], a_h[msel], a_w[msel]]).reshape(128, 1)
        locw_a = np.concatenate(
            [a_h[msel], a_w[msel],
             np.full(M_LOC, LN10, f32), np.full(M_LOC, LN10, f32)]).reshape(128, 1)
        in_maps.append({
            "masks": np.ascontiguousarray(gt_masks[G_LOC * cidx:G_LOC * (cidx + 1)]),
            "proto16": proto16,
            "proto_cat": proto_cat,
            "w16": w16,
            "sr": sr,
            "clsx": np.ascontiguousarray(clsx_a, dtype=f32),
            "clssgn": clssgn_a.astype(f32),
            "locp": locp_a.astype(f32),
            "locu": locu_a.astype(f32),
            "locv": np.ascontiguousarray(locv_a, dtype=f32),
            "locw": np.ascontiguousarray(locw_a, dtype=f32),
        })
    return in_maps, C


def _combine(results, C):
    """results: list of per-core {'res': [128, NCOL]} dicts. float64 combine."""
    s_soft = 0.0
    s_cls = 0.0
    s_loc = 0.0
    s_dot = 0.0
    for cidx in range(N_CORES):
        rc = np.asarray(results[cidx]["res"], np.float64)
        s_soft += rc[:, 0:8].sum()
        s_cls += rc[:, 8].sum()
        s_loc += rc[:, 9].sum()
        for g in range(G_LOC):
            for p in range(4):
                s_dot += C[p, G_LOC * cidx + g] * rc[:, 10 + 4 * g + p].sum()
    total = s_cls + s_loc + (16.0 * s_soft - s_dot) / 262144.0 / float(M)
    return np.array(total, dtype=np.float32)


def kernel(**inputs):
    from concourse.bass_utils import run_bass_kernel_spmd
    nc = _get_nc()
    in_maps, C = _host_prep(inputs)
    out = run_bass_kernel_spmd(nc, in_maps, list(range(N_CORES)))
    return _combine(out.results, C)


# revision 11
# speedup vs baseline: 1.2237x; 1.0250x over previous
"""Trainium2 Bass kernel for nn_AllLoss_13400297964003.

Strategy (exact algebraic refactor of the reference loss):
  - The mask BCE term per anchor m is
        mean_{512x512}( softplus(up) - goal*up )
    with up = 4x nearest-upsample of z_m = coef_m . proto.  This equals
        ( 16*sum_ij softplus(z_m[ij]) - sum_ij z_m[ij]*G_m[ij] ) / 512^2
    where G_m = 4x4 block-sum pooling of gt_masks[gt_idx[m]].
  - The goal term collapses:  sum_m sum_ij z_m*G_m = sum_{p,g} C[p,g]*D[p,g]
    with C[p,g] = sum_{m: gt_idx[m]=g} coef[m,p]  (tiny, host-aggregated)
    and  D[p,g] = sum_ij proto[p,ij] * pool4x4(mask_g)[ij]  (device).
  - Sharding over 8 cores: core c gets anchors [32c,32c+32), gt masks
    [4c,4c+4), and 96 negative anchors.  Each core reads 4.2MB of masks
    (a perfect shard of the 33.5MB dominant input), computes partial sums,
    host combines scalars in float64.

Device work per core:
  - z via float32r matmuls: block-diag weights [16,128] x proto16
    [16,4096] -> z in PSUM (full partition occupancy).  softplus = Exp
    then Ln(bias=1) on ACT with accum_out; all Exp-set ops are
    chain-ordered before all Ln-set ops so the ACT spline tables load
    exactly twice.
  - mask 4x4 pooling: row-pool via float32r matmuls with constant 0/1
    matrices (exact for 0/1 masks), column-pool via one strided
    tensor_reduce per mask.
  - D partials via DVE multiply + segmented reduce.
  - cls/loc losses as packed 128-row columns (gathers done host-side,
    all arithmetic incl. log10/reciprocal/smooth-L1 on device).
"""

import numpy as np

N_CORES = 8
M = 256
NUM_GT = 32
M_LOC = M // N_CORES          # 32 anchors per core
G_LOC = NUM_GT // N_CORES     # 4 gt masks per core
NEG_LOC = 3 * M // N_CORES    # 96 negative anchors per core
LN10 = float(np.log(10.0))
NCOL = 26                     # result cols: 8 soft, 1 cls, 1 loc, 16 ddot

_CACHE = {}


def _build_nc():
    from contextlib import ExitStack
    import concourse.tile as tile
    from concourse import bacc, mybir
    from concourse.tile import add_dep_helper

    f32 = mybir.dt.float32
    f32r = mybir.dt.float32r
    AF = mybir.ActivationFunctionType
    ALU = mybir.AluOpType
    AX = mybir.AxisListType

    nc = bacc.Bacc("TRN2", target_bir_lowering=False, debug=False)

    masks = nc.dram_tensor("masks", [G_LOC, 512, 512], f32r, kind="ExternalInput").ap()
    proto16 = nc.dram_tensor("proto16", [16, 4096], f32r, kind="ExternalInput").ap()
    proto_cat = nc.dram_tensor("proto_cat", [128, 512], f32, kind="ExternalInput").ap()
    w16 = nc.dram_tensor("w16", [16, 128], f32r, kind="ExternalInput").ap()
    sr = nc.dram_tensor("sr", [128, 512], f32r, kind="ExternalInput").ap()
    small8 = nc.dram_tensor("small8", [128, 8], f32, kind="ExternalInput").ap()
    res = nc.dram_tensor("res", [128, NCOL], f32, kind="ExternalOutput").ap()

    with tile.TileContext(nc) as tc:
        with ExitStack() as ctx:
            constp = ctx.enter_context(tc.tile_pool(name="constp", bufs=1))
            maskp = ctx.enter_context(tc.tile_pool(name="maskp", bufs=16))
            zps = ctx.enter_context(tc.tile_pool(name="zps", bufs=4, space="PSUM"))
            rps = ctx.enter_context(tc.tile_pool(name="rps", bufs=4, space="PSUM"))
            exps = ctx.enter_context(tc.tile_pool(name="exps", bufs=8))
            workp = ctx.enter_context(tc.tile_pool(name="workp", bufs=3))
            outp = ctx.enter_context(tc.tile_pool(name="outp", bufs=1))

            # ---- constant / small input loads ----
            proto16_t = constp.tile([16, 4096], f32r)
            nc.scalar.dma_start(proto16_t[:], proto16[:])
            w16_t = constp.tile([16, 128], f32r)
            nc.scalar.dma_start(w16_t[:], w16[:])
            sr_t = constp.tile([128, 512], f32r)
            nc.sync.dma_start(sr_t[:], sr[:])
            small8_t = constp.tile([128, 8], f32)
            nc.scalar.dma_start(small8_t[:], small8[:])
            clsx_t = small8_t[:, 0:1]
            clssgn_t = small8_t[:, 1:2]
            locp_t = small8_t[:, 2:3]
            locu_t = small8_t[:, 3:4]
            locv_t = small8_t[:, 4:5]
            locw_t = small8_t[:, 5:6]

            # ---- mask chunk DMAs (the dominant traffic) ----
            chunk = {}
            rings = [nc.sync, nc.scalar]
            for idx, (g, c) in enumerate(
                    [(g, c) for g in range(G_LOC) for c in range(4)]):
                t = maskp.tile([128, 512], f32r, tag="mask")
                rings[idx % 2].dma_start(t[:], masks[g, 128 * c:128 * (c + 1), :])
                chunk[(g, c)] = t

            proto_cat_t = constp.tile([128, 512], f32)
            nc.sync.dma_start(proto_cat_t[:], proto_cat[:])

            PS = outp.tile([128, NCOL], f32)

            exp_phase = []   # ACT ops using the Exp table set (+ fillers)
            ln_phase = []    # ACT ops using the Ln table set

            # ---- z matmuls (f32r) + softplus ----
            ex_tiles = []
            for b in range(8):
                zt = zps.tile([128, 512], f32, tag="z")
                nc.tensor.matmul(zt[:], w16_t[:],
                                 proto16_t[:, 512 * b:512 * (b + 1)],
                                 start=True, stop=True)
                ex = exps.tile([128, 512], f32, tag="ex")
                exp_phase.append(nc.scalar.activation(ex[:], zt[:], AF.Exp))
                ex_tiles.append(ex)

            # ---- cls/loc ACT ops, grouped into the two table phases ----
            et = workp.tile([128, 1], f32, tag="sm1")
            exp_phase.append(
                nc.scalar.activation(et[:], clsx_t, AF.Exp, scale=clssgn_t))
            fu = workp.tile([128, 1], f32, tag="sm2")
            exp_phase.append(
                nc.scalar.activation(fu[0:64, :], locu_t[0:64, :], AF.Identity))
            fv = workp.tile([128, 1], f32, tag="sm3")
            exp_phase.append(
                nc.scalar.activation(fv[0:64, :], locv_t[0:64, :], AF.Identity))

            for b in range(8):
                ln_phase.append(
                    nc.scalar.activation(ex_tiles[b][:], ex_tiles[b][:], AF.Ln,
                                         bias=1.0, accum_out=PS[:, b:b + 1]))
            ln_phase.append(
                nc.scalar.activation(PS[:, 8:9], et[:], AF.Ln, bias=1.0))
            ln_phase.append(
                nc.scalar.activation(fu[64:128, :], locu_t[64:128, :], AF.Ln))
            ln_phase.append(
                nc.scalar.activation(fv[64:128, :], locv_t[64:128, :], AF.Ln))

            # chain the ACT program order: all Exp-set ops, then all Ln-set ops
            order = exp_phase + ln_phase
            for a, b2 in zip(order, order[1:]):
                add_dep_helper(b2.ins, a.ins, sync=False, reason="act-table-phase")

            # ---- mask pooling + D partials ----
            dve_order = []
            pc3 = proto_cat_t[:].rearrange("p (a k) -> p a k", a=4)
            for g in range(G_LOC):
                R = rps.tile([128, 512], f32, tag="r")
                for c in range(4):
                    nc.tensor.matmul(
                        R[:],
                        sr_t[:, 128 * c:128 * (c + 1)],
                        chunk[(g, c)][:],
                        start=(c == 0), stop=(c == 3),
                    )
                r4 = R[:].rearrange("p (j four) -> p j four", four=4)
                Pg = workp.tile([128, 128], f32, tag="Pg")
                dve_order.append(
                    nc.vector.tensor_reduce(Pg[:], r4, axis=AX.X, op=ALU.add))
                prod = workp.tile([128, 4, 128], f32, tag="prod")
                pgb = Pg[:].unsqueeze(1).broadcast_to([128, 4, 128])
                dve_order.append(nc.vector.tensor_mul(prod[:], pgb, pc3))
                dve_order.append(
                    nc.vector.tensor_reduce(PS[:, 10 + 4 * g:14 + 4 * g], prod[:],
                                            axis=AX.X, op=ALU.add))

            # ---- localization smooth-L1 column ----
            rw = workp.tile([128, 1], f32, tag="sm4")
            nc.vector.reciprocal(rw[:], locw_t)
            for a, b2 in zip(dve_order, dve_order[1:]):
                add_dep_helper(b2.ins, a.ins, sync=False, reason="dve-order")
            last_pool = dve_order[-1]
            df = workp.tile([128, 1], f32, tag="sm5")
            df_i = nc.vector.tensor_sub(df[:], fu[:], fv[:])
            add_dep_helper(df_i.ins, last_pool.ins, sync=False, reason="loc-last")
            tgt = workp.tile([128, 1], f32, tag="sm6")
            nc.vector.tensor_mul(tgt[:], df[:], rw[:])
            d = workp.tile([128, 1], f32, tag="sm7")
            nc.vector.tensor_sub(d[:], locp_t, tgt[:])
            a_t = workp.tile([128, 1], f32, tag="sm8")
            nc.scalar.activation(a_t[:], d[:], AF.Abs)
            mn = workp.tile([128, 1], f32, tag="sm9")
            nc.vector.tensor_scalar(mn[:], a_t[:], 1.0, None, op0=ALU.min)
            amn = workp.tile([128, 1], f32, tag="sm10")
            nc.vector.tensor_sub(amn[:], a_t[:], mn[:])
            sq = workp.tile([128, 1], f32, tag="sm11")
            nc.vector.tensor_mul(sq[:], mn[:], mn[:])
            nc.vector.scalar_tensor_tensor(PS[:, 9:10], sq[:], 0.5, amn[:],
                                           op0=ALU.mult, op1=ALU.add)

            # ---- write result ----
            nc.sync.dma_start(res[:], PS[:])

    nc.compile()
    return nc


def _get_nc():
    if "nc" not in _CACHE:
        _CACHE["nc"] = _build_nc()
    return _CACHE["nc"]


def _host_prep(inputs):
    """Pure index-driven gathers/packing. Returns per-core input maps plus
    the float64 C aggregation matrix used in the final scalar combine."""
    f32 = np.float32
    proto = np.asarray(inputs["proto_types"], f32)[0]        # (4,128,128)
    map_class = np.asarray(inputs["map_class"], f32)[0]      # (3,64,64)
    map_box = np.asarray(inputs["map_box"], f32)[0]          # (12,64,64)
    map_coef = np.asarray(inputs["map_coef"], f32)[0]        # (12,64,64)
    anchor_center = np.asarray(inputs["anchor_center"], f32)  # (2,64,64)
    anchor_box = np.asarray(inputs["anchor_box"], f32)       # (3,2)
    gt_boxes = np.asarray(inputs["gt_boxes"], f32)[0]        # (32,4)
    gt_masks = np.asarray(inputs["gt_masks"], f32)[0]        # (32,512,512)
    pos_idx = np.asarray(inputs["pos_idx"])
    gt_idx = np.asarray(inputs["gt_idx"])
    neg_idx = np.asarray(inputs["neg_idx"])

    r, hh, ww = pos_idx[:, 0], pos_idx[:, 1], pos_idx[:, 2]
    ch4 = r[:, None] * 4 + np.arange(4, dtype=r.dtype)[None, :]
    coef = map_coef[ch4, hh[:, None], ww[:, None]]           # (256,4)
    pred = map_box[ch4, hh[:, None], ww[:, None]]            # (256,4)
    logit_pos = map_class[r, hh, ww]                         # (256,)
    logit_neg = map_class[neg_idx[:, 0], neg_idx[:, 1], neg_idx[:, 2]]  # (768,)
    a_ch = anchor_center[0, hh, ww]
    a_cw = anchor_center[1, hh, ww]
    a_h = anchor_box[r, 0]
    a_w = anchor_box[r, 1]
    gt = gt_boxes[gt_idx]                                    # (256,4)

    # replicated tensors
    proto_flat = proto.reshape(4, 16384)
    proto16 = np.ascontiguousarray(
        proto_flat.reshape(4, 4, 4096).transpose(1, 0, 2).reshape(16, 4096))
    proto_cat = np.ascontiguousarray(proto.transpose(1, 0, 2).reshape(128, 512))
    sr = np.zeros((128, 512), f32)
    for c in range(4):
        for I in range(128):
            sr[I, 128 * c + 32 * c + I // 4] = 1.0

    # C[p,g] aggregation (float64, host)
    C = np.zeros((4, NUM_GT), np.float64)
    for p in range(4):
        np.add.at(C[p], gt_idx, coef[:, p].astype(np.float64))

    in_maps = []
    for cidx in range(N_CORES):
        msel = slice(M_LOC * cidx, M_LOC * (cidx + 1))
        nsel = slice(NEG_LOC * cidx, NEG_LOC * (cidx + 1))
        coef_c = coef[msel]                                  # (32,4)
        w16 = np.zeros((16, 128), f32)
        for q in range(4):
            w16[4 * q:4 * q + 4, 32 * q:32 * q + 32] = coef_c.T
        small = np.zeros((128, 8), f32)
        small[:, 0] = np.concatenate([logit_pos[msel], logit_neg[nsel]])
        small[:, 1] = np.concatenate(
            [np.full(M_LOC, -1.0, f32), np.full(NEG_LOC, 1.0, f32)])
        # k-blocked loc packing: rows k*32 + j
        small[:, 2] = pred[msel].T.reshape(128)
        small[:, 3] = gt[msel].T.reshape(128)
        small[:, 4] = np.concatenate(
            [a_ch[msel], a_cw[msel], a_h[msel], a_w[msel]])
        small[:, 5] = np.concatenate(
            [a_h[msel], a_w[msel],
             np.full(M_LOC, LN10, f32), np.full(M_LOC, LN10, f32)])
        in_maps.append({
            "masks": np.ascontiguousarray(gt_masks[G_LOC * cidx:G_LOC * (cidx + 1)]),
            "proto16": proto16,
            "proto_cat": proto_cat,
            "w16": w16,
            "sr": sr,
            "small8": small,
        })
    return in_maps, C


def _combine(results, C):
    """results: list of per-core {'res': [128, NCOL]} dicts. float64 combine."""
    s_soft = 0.0
    s_cls = 0.0
    s_loc = 0.0
    s_dot = 0.0
    for cidx in range(N_CORES):
        rc = np.asarray(results[cidx]["res"], np.float64)
        s_soft += rc[:, 0:8].sum()
        s_cls += rc[:, 8].sum()
        s_loc += rc[:, 9].sum()
        for g in range(G_LOC):
            for p in range(4):
                s_dot += C[p, G_LOC * cidx + g] * rc[:, 10 + 4 * g + p].sum()
    total = s_cls + s_loc + (16.0 * s_soft - s_dot) / 262144.0 / float(M)
    return np.array(total, dtype=np.float32)


def kernel(**inputs):
    from concourse.bass_utils import run_bass_kernel_spmd
    nc = _get_nc()
    in_maps, C = _host_prep(inputs)
    out = run_bass_kernel_spmd(nc, in_maps, list(range(N_CORES)))
    return _combine(out.results, C)
